# revision 25
# baseline (speedup 1.0000x reference)
"""Bahdanau attention on 8 Trainium2 NeuronCores (Bass/Tile).

Problem:  S=2048, B=32, D=1024, E2=1024
  ws  = dec @ Wb.T                       [B, D]
  WH  = enc @ Wc.T                       [S, B, D]
  sc  = tanh(WH + ws) . Wa               [S, B]
  at  = softmax(sc, axis=0)              [S, B]
  out = einsum('sb,sbe->be', at, enc)[None]   [1, B, 2E]

Sharding: data-parallel over batch B across 8 cores (4 batches/core);
Wb/Wc/Wa replicated. Softmax axis (S) stays core-local.

The WH matmul (17.2 GFLOP/core) is the PE bottleneck. It runs as fp8e4
DoubleRow passes, each contracting TWO 128-e chunks in the time a bf16
matmul contracts one (measured ~216ns per 512-col pass either way), so
the WH instruction stream halves: 32 passes per (batch, s-tile) vs 64.

Plain nearest-rounded fp8 on both operands lands at rel-err 2.2e-2,
over the 2e-2 budget: the ~6%/element e4m3 noise becomes ~0.025
score noise which the softmax turns into context error. Fix: host-side
WEIGHTED ERROR-CANCELLING QUANTIZATION. The score only feels rounding
error through weighted sums -- sum_d Wa_d * dWc[e,d] per e-row of Wc,
and sum_e v_e * denc[s,b,e] (v = Wc_q @ Wa) per (s,b) row of enc. After
nearest rounding, ~21 near-tie elements per row are flipped to their
adjacent fp8 grid point (greedy subset-sum) so those weighted error
sums cancel to ~0. This removes the score-aligned noise component at
~zero per-element cost: measured rel-err 1.24e-2 on the target inputs.
The fp8 scales (enc*4, Wc*32) dodge e4m3 subnormals; the /128 descale
is folded into the tanh activation's scale.

All other tensors are bf16 (enc natural layout for the context matmul,
Wb/Wa/dec), halving DMA vs fp32. Numerics: tanh output bf16, exp row
f16, context accumulated fp32 in PSUM and normalized by 1/Z at the end
(no max-subtraction: |score| <= ~5 for these inputs, exp fits f16).

Per-core dataflow per (batch, s-tile):
  - WH^T [d-chunk=128p, s'] = 4 fp8 DoubleRow passes (8 e-chunks) in PSUM
  - ACT: tanh(WH/128 + wsT[d,b]) via per-partition bias, scale=1/128
  - score row [1, s'] = sum_d WaChunk.T @ tanh_chunk  (PE, bf16)
  - exp on ACT, Z partials via activation accum_out
  - exp row -> column via a K=32 matmul against e0 (rows 1-31 zeroed)
  - context [1, e] += expCol.T @ enc_nat(bf16) on PE, scaled 1/Z per batch

Engines run their instruction streams in order, so emission order doubles
as a schedule: tile(0,0)'s enc DMAs are emitted before the bulky weight
DMAs to cover DMA latency at kernel start; the (columnize + ctx) block of
tile t is emitted after tile t+1's WH/score work so the PE never stalls
on ACT's exp.
"""

import numpy as np

S, B, D, E2 = 2048, 32, 1024, 1024
NCORES = 8
BL = B // NCORES          # batches per core
ST = 512                  # s-tile size
NST = S // ST             # s-tiles per batch
NSUB = ST // 128          # 128-row subtiles per s-tile
DC = D // 128             # d chunks
EC2 = E2 // 256           # e chunk-pairs (one DoubleRow pass each)

SC_E = 4.0                # fp8 enc pre-scale (power of 2, exact)
SC_W = 32.0               # fp8 Wc pre-scale
DESCALE = 1.0 / (SC_E * SC_W)
TIE = 0.45                # flip only near-tie roundings (|err| >= TIE*step)

_CACHE = {}


def _build_nc():
    import concourse.bacc as bacc
    import concourse.tile as tile
    from concourse import mybir
    from concourse.masks import make_identity

    f32 = mybir.dt.float32
    f32r = mybir.dt.float32r
    f16 = mybir.dt.float16
    bf16 = mybir.dt.bfloat16
    f8 = mybir.dt.float8e4
    DR = mybir.MatmulPerfMode.DoubleRow
    TANH = mybir.ActivationFunctionType.Tanh
    EXP = mybir.ActivationFunctionType.Exp
    X = mybir.AxisListType.X

    nc = bacc.Bacc()
    # enct8[b, p, c, i, s] = fp8q(enc[s, b, 256c+128i+p] * 4)
    enct8 = nc.declare_dram_parameter("enct8", [BL, 128, EC2, 2, S], f8, isOutput=False)
    encn = nc.declare_dram_parameter("encn", [S, BL, E2], bf16, isOutput=False)
    dect = nc.declare_dram_parameter("dect", [128, DC, BL], bf16, isOutput=False)
    wbt = nc.declare_dram_parameter("wbt", [D, D], bf16, isOutput=False)     # Wb.T
    # wct8[p, c, i, d] = fp8q(Wc.T[256c+128i+p, d] * 32)
    wct8 = nc.declare_dram_parameter("wct8", [128, EC2, 2, D], f8, isOutput=False)
    wa2 = nc.declare_dram_parameter("wa2", [128, DC], bf16, isOutput=False)  # Wa chunks
    outp = nc.declare_dram_parameter("out", [BL, E2], f32, isOutput=True)

    with tile.TileContext(nc) as tc:
        with (
            tc.tile_pool(name="const", bufs=1) as const_pool,
            tc.tile_pool(name="wbtp", bufs=1) as wbt_pool,
            tc.tile_pool(name="encn", bufs=3) as encn_pool,
            tc.tile_pool(name="enct", bufs=3) as enct_pool,
            tc.tile_pool(name="tanhp", bufs=4) as tanh_pool,
            tc.tile_pool(name="rows", bufs=2) as row_pool,
            tc.tile_pool(name="wh_ps", bufs=4, space="PSUM") as wh_ps,
            tc.tile_pool(name="sc_ps", bufs=2, space="PSUM") as sc_ps,
            tc.tile_pool(name="ctx_ps", bufs=2, space="PSUM") as ctx_ps,
        ):
            ex_ps = wh_ps  # columnize rides the wh rotation (same tag below)

            # ---- identity (fp32 via gpsimd) + fp16 e0 for the columnize ----
            id32 = const_pool.tile([128, 128], f32)
            make_identity(nc, id32)
            e0f16 = const_pool.tile([32, NSUB], f16)
            nc.vector.tensor_copy(out=e0f16, in_=id32[0:32, 0:NSUB])

            # ---- PE clock warm-up ----
            # The PE ramps 0.65 -> 1.2 -> 2.4 GHz over ~3us of continuous
            # execution. Burn that ramp on dummy fp32 matmuls (4 cyc/row)
            # during the otherwise-idle startup DMA window so the ws chain
            # and tile(0,0) run at full clock.
            warm_ps = wh_ps.tile([128, ST], f32, tag="wh", name="warm")
            for _ in range(6):
                nc.tensor.matmul(warm_ps[:, 0:128], id32, id32,
                                 start=True, stop=True)

            # ---- emission helpers (order == per-engine schedule) ----
            # One fused dma_start per tile load: SWDGE descriptor generation
            # costs ~0.6us per dma_start on the GpSimd Q7, so batch the
            # subtiles into a single transfer with a 3/4-D access pattern.
            def load_subs(bj, st, pfx=""):
                s0 = st * ST
                sub_big = encn_pool.tile([128, NSUB, E2], bf16, tag="sub",
                                         name=f"sub{pfx}")
                nc.gpsimd.dma_start(
                    out=sub_big,
                    in_=encn[s0:s0 + ST, bj, :].rearrange("(j p) e -> p j e", p=128),
                )
                return sub_big

            def load_enct(bj, st, pfx=""):
                s0 = st * ST
                et8 = enct_pool.tile([128, EC2, 2, ST], f8, tag="et8", bufs=3,
                                     name=f"et8{pfx}")
                nc.gpsimd.dma_start(out=et8, in_=enct8[bj, :, :, :, s0:s0 + ST])
                return et8

            # DMA queue order tuned for startup, strictly by first PE need:
            # ws inputs (dect tiny, then wbt as one fused load), then the
            # WH-path tile(0,0) load, then Wc^T, then the ctx-path tile(0,0).
            dect_sb = const_pool.tile([128, DC, BL], bf16)
            nc.gpsimd.dma_start(out=dect_sb, in_=dect[:, :, :])
            # wbt in 4 chunked loads of 2 d-blocks: the first ws matmul only
            # waits ~512KB, and later chunks land while the PE consumes
            # earlier ones.
            wbt_sb = []
            for g in range(4):
                t = wbt_pool.tile([128, 2, D], bf16, tag="wbt_sb", bufs=4,
                                  name=f"wbt{g}")
                nc.gpsimd.dma_start(
                    out=t,
                    in_=wbt[g * 256:(g + 1) * 256, :]
                    .rearrange("(c p) d -> p c d", p=128),
                )
                wbt_sb.append(t)
            enct_cache = {(0, 0): load_enct(0, 0, pfx="00")}
            wct8_sb = const_pool.tile([128, EC2, 2, D], f8)
            nc.gpsimd.dma_start(out=wct8_sb, in_=wct8[:, :, :, :])
            subs_cache = {(0, 0): load_subs(0, 0, pfx="00")}
            wa_sb = const_pool.tile([128, DC], bf16)
            nc.gpsimd.dma_start(out=wa_sb, in_=wa2[:, :])

            # ---- ws = dec @ Wb.T -> wsT [d2-chunk, b] for the tanh bias ----
            # Emitted AFTER tile(0,0)'s WH DoubleRow groups (see emit_wh) so
            # the PE crunches tile(0,0) while the ws-path DMAs land. The ws
            # psum rides the ctx pool and the transpose scratch the sc pool:
            # all four wh-pool bufs hold tile(0,0)'s un-tanh'ed output at
            # that point, and tanh can't run before ws -> same-pool
            # allocation would deadlock the PE stream.
            ws_sb = const_pool.tile([BL, D], f32r)
            id32r = const_pool.tile([BL, BL], f32r)
            nc.vector.tensor_copy(out=id32r, in_=id32[0:BL, 0:BL])
            wst_sb = []

            def emit_ws():
                ws_psum = [ctx_ps.tile([BL, 512], f32, tag="ctx", name=f"ws_psum{eh}")
                           for eh in range(2)]
                for dk in range(DC):
                    for eh in range(2):
                        nc.tensor.matmul(
                            ws_psum[eh], dect_sb[:, dk, :],
                            wbt_sb[dk // 2][:, dk % 2, eh * 512:(eh + 1) * 512],
                            start=(dk == 0), stop=(dk == DC - 1),
                        )
                for eh in range(2):
                    nc.scalar.copy(out=ws_sb[:, eh * 512:(eh + 1) * 512],
                                   in_=ws_psum[eh])
                # transpose ws -> wst chunks [128, BL] (tiny, K=BL transpose)
                for dcn in range(DC):
                    tp = sc_ps.tile([128, ST], f32r, tag="sc", name="tp_ws")
                    nc.tensor.transpose(
                        tp[:, 0:BL], ws_sb[0:BL, dcn * 128:(dcn + 1) * 128], id32r
                    )
                    w = const_pool.tile([128, BL], f32, tag="wst_sb", bufs=DC,
                                        name=f"wst{dcn}")
                    nc.vector.tensor_copy(out=w, in_=tp[:, 0:BL])
                    wst_sb.append(w)

            # ---- main loop over (batch, s-tile) ----
            # The (exp-columnize + ctx) block of tile t is emitted after tile
            # t+1's WH/score work: the PE would otherwise idle ~1us per tile
            # waiting for ACT's exp. `pending` carries tile t's closure.
            state = {}   # per-b: exp_all, zparts, ctx
            pending = [] # [(bj, st, sub_big)]

            def emit_ctx(bj, st, sub_big):
                s0 = st * ST
                exp_all = state[bj]["exp_all"]
                # one matmul yields all NSUB exp columns: rows 0..3 of
                # exp_all hold the exp row at 128-element shifts (written by
                # ACT below), so lhsT[n, m] = exp[s0+128n+m] and the basis
                # rhs e0f16 picks ex[:, n] = column n.
                ex = ex_ps.tile([128, NSUB], f32, tag="wh", name="ex")
                nc.tensor.matmul(
                    ex, exp_all[0:32, s0:s0 + 128], e0f16,
                    start=True, stop=True,
                )
                ext = row_pool.tile([128, NSUB], bf16, tag="ext", bufs=3)
                nc.vector.tensor_copy(out=ext, in_=ex)
                # per-tile psum group, folded into the SBUF accumulator by DVE
                ctx_acc = state[bj]["ctx_acc"]
                for eh in range(2):
                    ctx_t = ctx_ps.tile([1, 512], f32, tag="ctx", name="ctx_t")
                    for j in range(NSUB):
                        nc.tensor.matmul(
                            ctx_t,
                            ext[:, j:j + 1],
                            sub_big[:, j, eh * 512:(eh + 1) * 512],
                            start=(j == 0), stop=(j == NSUB - 1),
                        )
                    sl = ctx_acc[0:1, eh * 512:(eh + 1) * 512]
                    nc.vector.tensor_add(out=sl, in0=sl, in1=ctx_t)

            def finish_batch(bj):
                z = row_pool.tile([1, 1], f32, tag="z")
                nc.vector.reduce_sum(out=z, in_=state[bj]["zparts"], axis=X)
                rz = row_pool.tile([1, 1], f32, tag="rz")
                nc.vector.reciprocal(out=rz, in_=z)
                ctx_sb = row_pool.tile([1, E2], f32, tag="ctx_sb")
                nc.vector.tensor_scalar_mul(
                    out=ctx_sb, in0=state[bj]["ctx_acc"], scalar1=rz,
                )
                nc.sync.dma_start(out=outp[bj:bj + 1, :], in_=ctx_sb)

            emit_ws()

            for bj in range(BL):
                # [32, S] so the row->column move can be a K=32 matmul against
                # e0 (rows 1-31 are zero); only row 0 holds exp scores.
                # fp16 is safe here: scores are bounded well below fp16's
                # exp-overflow point (|score| <= ~5 for randn-scale inputs,
                # overflow needs >11).
                exp_all = row_pool.tile([32, S], f16, tag="exp_all")
                nc.vector.memset(exp_all, 0.0)
                zparts = row_pool.tile([1, NST], f32, tag="zparts")
                ctx_acc = row_pool.tile([1, E2], f32, tag="ctx_acc")
                nc.vector.memset(ctx_acc, 0.0)
                state[bj] = dict(exp_all=exp_all, zparts=zparts, ctx_acc=ctx_acc)

                for st in range(NST):
                    s0 = st * ST
                    sub_big = subs_cache.pop((bj, st), None)
                    if sub_big is None:
                        sub_big = load_subs(bj, st)
                    et8 = enct_cache.pop((bj, st), None)
                    if et8 is None:
                        et8 = load_enct(bj, st)

                    # WH^T + tanh + score, d-chunks in pairs. The c-loop is
                    # outermost so consecutive matmuls alternate PSUM banks
                    # and re-read the same et8 chunk while it's hot; the
                    # tanh+score block of pair dp is deferred until after
                    # pair dp+1's DR groups so the PE never waits on ACT.
                    def emit_dr_pair(dp):
                        whs = [wh_ps.tile([128, ST], f32, tag="wh", name=f"wh{dd}")
                               for dd in range(2)]
                        for c in range(EC2):
                            for dd in range(2):
                                dcn = dp * 2 + dd
                                nc.tensor.matmul(
                                    whs[dd],
                                    wct8_sb[:, c, :, dcn * 128:(dcn + 1) * 128],
                                    et8[:, c, :, :],
                                    start=(c == 0), stop=(c == EC2 - 1),
                                    perf_mode=DR,
                                )
                        return whs

                    def emit_tanh_score(dp, whs, sc):
                        for dd in range(2):
                            dcn = dp * 2 + dd
                            th = tanh_pool.tile([128, ST], bf16, tag="th", name="th")
                            nc.scalar.activation(
                                out=th, in_=whs[dd], func=TANH,
                                bias=wst_sb[dcn][:, bj:bj + 1], scale=DESCALE,
                            )
                            nc.tensor.matmul(
                                sc, wa_sb[:, dcn:dcn + 1], th,
                                start=(dcn == 0), stop=(dcn == DC - 1),
                            )

                    sc = sc_ps.tile([1, ST], f32, tag="sc")
                    prev = None
                    for dp in range(DC // 2):
                        whs = emit_dr_pair(dp)
                        if prev is not None:
                            emit_tanh_score(dp - 1, prev, sc)
                        prev = whs
                    emit_tanh_score(DC // 2 - 1, prev, sc)

                    # exp (+ per-tile partial of Z via accum_out), then rows
                    # 1..3 get the 128-shifted head of the same exp values so
                    # the columnize collapses to a single K=32 matmul. ACT
                    # can't write at partition base 1 (BIR alignment), so the
                    # copies ride tiny SBUF->SBUF DMAs, hidden by the
                    # one-tile ctx deferral.
                    nc.scalar.activation(
                        out=exp_all[0:1, s0:s0 + ST], in_=sc, func=EXP,
                        accum_out=zparts[0:1, st:st + 1],
                    )
                    for r in range(1, NSUB):
                        nc.gpsimd.dma_start(
                            out=exp_all[r:r + 1, s0:s0 + 128],
                            in_=exp_all[0:1, s0 + r * 128:s0 + (r + 1) * 128],
                        )

                    # deferred ctx of the previous tile
                    if pending:
                        emit_ctx(*pending.pop())
                    pending.append((bj, st, sub_big))

                    if st == NST - 1 and bj > 0:
                        # previous batch is fully accumulated once its last
                        # pending ctx ran (one tile ago) -> normalize + store
                        finish_batch(bj - 1)

            emit_ctx(*pending.pop())
            finish_batch(BL - 1)

    nc.finalize()
    return nc


def _cancel_quant(x, a, tie=TIE, topk=256):
    """Quantize x [rows, K] to fp8e4 such that the a-weighted rounding error
    sum_k a_k*(q-x)_k cancels to ~0 per row. Nearest rounding, then a greedy
    subset-sum over near-tie elements flips them to the adjacent grid point."""
    import ml_dtypes

    f8 = ml_dtypes.float8_e4m3
    x32 = np.ascontiguousarray(x, dtype=np.float32)
    a32 = np.asarray(a, dtype=np.float32)
    qn = x32.astype(f8)
    qnf = qn.astype(np.float32)
    qf = (2.0 * x32 - qnf).astype(f8)      # reflect across x -> adjacent grid pt
    qff = qf.astype(np.float32)
    dq = qnf - x32
    step = np.abs(qff - qnf)
    c = np.where(np.abs(dq) >= tie * step, (qff - qnf) * a32[None, :], 0.0)
    A = dq @ a32
    idx = np.argpartition(-np.abs(c), topk - 1, axis=1)[:, :topk]
    csel = np.take_along_axis(c, idx, axis=1)
    ord2 = np.argsort(-np.abs(csel), axis=1)
    idx = np.take_along_axis(idx, ord2, axis=1)
    csel = np.take_along_axis(csel, ord2, axis=1)
    R = A.copy()
    take = np.zeros(csel.shape, dtype=bool)
    for k in range(topk):
        ck = csel[:, k]
        better = np.abs(R + ck) < np.abs(R)
        R += np.where(better, ck, 0.0)
        take[:, k] = better
    flip = np.zeros(c.shape, dtype=bool)
    np.put_along_axis(flip, idx, take, axis=1)
    return np.where(flip, qf, qn)


def _prep_inputs(dec_prev_hidden, enc_outputs, Wb, Wc, Wa):
    import ml_dtypes

    bf16 = ml_dtypes.bfloat16

    dec_prev_hidden = np.asarray(dec_prev_hidden, dtype=np.float32)
    enc_outputs = np.ascontiguousarray(np.asarray(enc_outputs, dtype=np.float32))
    Wb = np.asarray(Wb, dtype=np.float32)
    Wc = np.asarray(Wc, dtype=np.float32)
    Wa = np.asarray(Wa, dtype=np.float32)

    wbt16 = np.ascontiguousarray(Wb.T).astype(bf16)              # [d, d2]
    # error-cancelling fp8 quantization: Wc rows weighted by Wa, enc rows
    # weighted by v = Wc_q @ Wa (the Wa-weighted column sums).
    wq = _cancel_quant(np.ascontiguousarray(Wc.T) * SC_W, Wa)    # [e, d] fp8
    v = (wq.astype(np.float32) @ Wa) / SC_W                      # [e]
    xq = _cancel_quant(
        (enc_outputs * SC_E).reshape(S * B, E2), v
    ).reshape(S, B, E2)                                          # fp8
    wct8 = np.ascontiguousarray(
        wq.reshape(EC2, 2, 128, D).transpose(2, 0, 1, 3)         # [128, EC2, 2, D]
    )
    dect = dec_prev_hidden.T                                     # [D, B]
    wa16 = np.ascontiguousarray(Wa.reshape(DC, 128).T).astype(bf16)  # [128, DC]

    in_maps = []
    for i in range(NCORES):
        bsl = slice(i * BL, (i + 1) * BL)
        shard = enc_outputs[:, bsl, :]                           # [S, BL, E2]
        enct8 = np.ascontiguousarray(
            xq[:, bsl, :]
            .transpose(2, 1, 0)                                  # [E2, BL, S]
            .reshape(EC2, 2, 128, BL, S)
            .transpose(3, 2, 0, 1, 4)                            # [BL,128,EC2,2,S]
        )
        dect_c = np.ascontiguousarray(
            dect[:, bsl].reshape(DC, 128, BL).transpose(1, 0, 2)
        ).astype(bf16)                                           # [128, DC, BL]
        in_maps.append({
            "enct8": enct8,
            "encn": np.ascontiguousarray(shard).astype(bf16),
            "dect": dect_c,
            "wbt": wbt16,
            "wct8": wct8,
            "wa2": wa16,
        })
    return in_maps


def _run(inputs, trace=False):
    from concourse.bass_utils import run_bass_kernel_spmd

    if "nc" not in _CACHE:
        _CACHE["nc"] = _build_nc()
    nc = _CACHE["nc"]
    in_maps = _prep_inputs(**inputs)
    res = run_bass_kernel_spmd(nc, in_maps, list(range(NCORES)), trace=trace)
    out = np.concatenate([res.results[i]["out"] for i in range(NCORES)], axis=0)
    return out[None, :, :].astype(np.float32), res


def kernel(dec_prev_hidden, enc_outputs, Wb, Wc, Wa):
    out, _ = _run(dict(
        dec_prev_hidden=dec_prev_hidden, enc_outputs=enc_outputs,
        Wb=Wb, Wc=Wc, Wa=Wa,
    ))
    return out


# revision 29
# speedup vs baseline: 1.0179x; 1.0179x over previous
"""Bahdanau attention on 8 Trainium2 NeuronCores (Bass/Tile).

Problem:  S=2048, B=32, D=1024, E2=1024
  ws  = dec @ Wb.T                       [B, D]
  WH  = enc @ Wc.T                       [S, B, D]
  sc  = tanh(WH + ws) . Wa               [S, B]
  at  = softmax(sc, axis=0)              [S, B]
  out = einsum('sb,sbe->be', at, enc)[None]   [1, B, 2E]

Sharding: data-parallel over batch B across 8 cores (4 batches/core);
Wb/Wc/Wa replicated. Softmax axis (S) stays core-local.

The WH matmul (17.2 GFLOP/core) is the PE bottleneck. It runs as fp8e4
DoubleRow passes, each contracting TWO 128-e chunks in the time a bf16
matmul contracts one (measured ~216ns per 512-col pass either way), so
the WH instruction stream halves: 32 passes per (batch, s-tile) vs 64.

Plain nearest-rounded fp8 on both operands lands at rel-err 2.2e-2,
over the 2e-2 budget: the ~6%/element e4m3 noise becomes ~0.025
score noise which the softmax turns into context error. Fix: host-side
WEIGHTED ERROR-CANCELLING QUANTIZATION. The score only feels rounding
error through weighted sums -- sum_d Wa_d * dWc[e,d] per e-row of Wc,
and sum_e v_e * denc[s,b,e] (v = Wc_q @ Wa) per (s,b) row of enc. After
nearest rounding, ~21 near-tie elements per row are flipped to their
adjacent fp8 grid point (greedy subset-sum) so those weighted error
sums cancel to ~0. This removes the score-aligned noise component at
~zero per-element cost: measured rel-err 1.24e-2 on the target inputs.
The fp8 scales (enc*4, Wc*32) dodge e4m3 subnormals; the /128 descale
is folded into the tanh activation's scale.

All other tensors are bf16 (enc natural layout for the context matmul,
Wb/Wa/dec), halving DMA vs fp32. Numerics: tanh output bf16, exp row
f16, context accumulated fp32 in PSUM and normalized by 1/Z at the end
(no max-subtraction: |score| <= ~5 for these inputs, exp fits f16).

Per-core dataflow per (batch, s-tile):
  - WH^T [d-chunk=128p, s'] = 4 fp8 DoubleRow passes (8 e-chunks) in PSUM
  - ACT: tanh(WH/128 + wsT[d,b]) via per-partition bias, scale=1/128
  - score row [1, s'] = sum_d WaChunk.T @ tanh_chunk  (PE, bf16)
  - exp on ACT, Z partials via activation accum_out
  - exp row -> column via a K=32 matmul against e0 (rows 1-31 zeroed)
  - context [1, e] += expCol.T @ enc_nat(bf16) on PE, scaled 1/Z per batch

Engines run their instruction streams in order, so emission order doubles
as a schedule: tile(0,0)'s enc DMAs are emitted before the bulky weight
DMAs to cover DMA latency at kernel start; the (columnize + ctx) block of
tile t is emitted after tile t+1's WH/score work so the PE never stalls
on ACT's exp.
"""

import numpy as np

S, B, D, E2 = 2048, 32, 1024, 1024
NCORES = 8
BL = B // NCORES          # batches per core
ST = 512                  # s-tile size
NST = S // ST             # s-tiles per batch
NSUB = ST // 128          # 128-row subtiles per s-tile
DC = D // 128             # d chunks
EC2 = E2 // 256           # e chunk-pairs (one DoubleRow pass each)

SC_E = 4.0                # fp8 enc pre-scale (power of 2, exact)
SC_W = 32.0               # fp8 Wc pre-scale
DESCALE = 1.0 / (SC_E * SC_W)
TIE = 0.45                # flip only near-tie roundings (|err| >= TIE*step)

_CACHE = {}


def _build_nc():
    import concourse.bacc as bacc
    import concourse.tile as tile
    from concourse import mybir
    from concourse.masks import make_identity

    f32 = mybir.dt.float32
    f32r = mybir.dt.float32r
    f16 = mybir.dt.float16
    bf16 = mybir.dt.bfloat16
    f8 = mybir.dt.float8e4
    DR = mybir.MatmulPerfMode.DoubleRow
    TANH = mybir.ActivationFunctionType.Tanh
    EXP = mybir.ActivationFunctionType.Exp
    X = mybir.AxisListType.X

    nc = bacc.Bacc()
    # enct8[b, p, c, i, s] = fp8q(enc[s, b, 256c+128i+p] * 4)
    enct8 = nc.declare_dram_parameter("enct8", [BL, 128, EC2, 2, S], f8, isOutput=False)
    encn = nc.declare_dram_parameter("encn", [S, BL, E2], bf16, isOutput=False)
    dect = nc.declare_dram_parameter("dect", [128, DC, BL], bf16, isOutput=False)
    wbt = nc.declare_dram_parameter("wbt", [D, D], bf16, isOutput=False)     # Wb.T
    # wct8[p, c, i, d] = fp8q(Wc.T[256c+128i+p, d] * 32)
    wct8 = nc.declare_dram_parameter("wct8", [128, EC2, 2, D], f8, isOutput=False)
    wa2 = nc.declare_dram_parameter("wa2", [128, DC], bf16, isOutput=False)  # Wa chunks
    outp = nc.declare_dram_parameter("out", [BL, E2], f32, isOutput=True)

    with tile.TileContext(nc) as tc:
        with (
            tc.tile_pool(name="const", bufs=1) as const_pool,
            tc.tile_pool(name="wbtp", bufs=1) as wbt_pool,
            tc.tile_pool(name="encn", bufs=3) as encn_pool,
            tc.tile_pool(name="enct", bufs=3) as enct_pool,
            tc.tile_pool(name="tanhp", bufs=4) as tanh_pool,
            tc.tile_pool(name="rows", bufs=2) as row_pool,
            tc.tile_pool(name="wh_ps", bufs=4, space="PSUM") as wh_ps,
            tc.tile_pool(name="sc_ps", bufs=2, space="PSUM") as sc_ps,
            tc.tile_pool(name="ctx_ps", bufs=2, space="PSUM") as ctx_ps,
        ):
            ex_ps = wh_ps  # columnize rides the wh rotation (same tag below)

            # ---- identity (fp32 via gpsimd) + fp16 e0 for the columnize ----
            id32 = const_pool.tile([128, 128], f32)
            make_identity(nc, id32)
            e0f16 = const_pool.tile([32, NSUB], f16)
            nc.vector.tensor_copy(out=e0f16, in_=id32[0:32, 0:NSUB])

            # ---- PE clock warm-up ----
            # The PE ramps 0.65 -> 1.2 -> 2.4 GHz over ~3us of continuous
            # execution. Burn that ramp on dummy fp32 matmuls (4 cyc/row)
            # during the otherwise-idle startup DMA window so the ws chain
            # and tile(0,0) run at full clock.
            warm_ps = wh_ps.tile([128, ST], f32, tag="wh", name="warm")
            for _ in range(6):
                nc.tensor.matmul(warm_ps[:, 0:128], id32, id32,
                                 start=True, stop=True)

            # ---- emission helpers (order == per-engine schedule) ----
            # One fused dma_start per tile load: SWDGE descriptor generation
            # costs ~0.6us per dma_start on the GpSimd Q7, so batch the
            # subtiles into a single transfer with a 3/4-D access pattern.
            def load_subs(bj, st, pfx=""):
                s0 = st * ST
                sub_big = encn_pool.tile([128, NSUB, E2], bf16, tag="sub",
                                         name=f"sub{pfx}")
                nc.gpsimd.dma_start(
                    out=sub_big,
                    in_=encn[s0:s0 + ST, bj, :].rearrange("(j p) e -> p j e", p=128),
                )
                return sub_big

            def load_enct(bj, st, pfx=""):
                s0 = st * ST
                et8 = enct_pool.tile([128, EC2, 2, ST], f8, tag="et8", bufs=3,
                                     name=f"et8{pfx}")
                nc.gpsimd.dma_start(out=et8, in_=enct8[bj, :, :, :, s0:s0 + ST])
                return et8

            # DMA queue order tuned for startup, strictly by first PE need:
            # ws inputs (dect tiny, then wbt as one fused load), then the
            # WH-path tile(0,0) load, then Wc^T, then the ctx-path tile(0,0).
            dect_sb = const_pool.tile([128, DC, BL], bf16)
            nc.gpsimd.dma_start(out=dect_sb, in_=dect[:, :, :])
            # wbt in 4 chunked loads of 2 d-blocks: the first ws matmul only
            # waits ~512KB, and later chunks land while the PE consumes
            # earlier ones.
            wbt_sb = []
            for g in range(4):
                t = wbt_pool.tile([128, 2, D], bf16, tag="wbt_sb", bufs=4,
                                  name=f"wbt{g}")
                nc.gpsimd.dma_start(
                    out=t,
                    in_=wbt[g * 256:(g + 1) * 256, :]
                    .rearrange("(c p) d -> p c d", p=128),
                )
                wbt_sb.append(t)
            enct_cache = {(0, 0): load_enct(0, 0, pfx="00")}
            wct8_sb = const_pool.tile([128, EC2, 2, D], f8)
            nc.gpsimd.dma_start(out=wct8_sb, in_=wct8[:, :, :, :])
            subs_cache = {(0, 0): load_subs(0, 0, pfx="00")}
            wa_sb = const_pool.tile([128, DC], bf16)
            nc.gpsimd.dma_start(out=wa_sb, in_=wa2[:, :])

            # ---- ws = dec @ Wb.T -> wsT [d2-chunk, b] for the tanh bias ----
            # Emitted AFTER tile(0,0)'s WH DoubleRow groups (see emit_wh) so
            # the PE crunches tile(0,0) while the ws-path DMAs land. The ws
            # psum rides the ctx pool and the transpose scratch the sc pool:
            # all four wh-pool bufs hold tile(0,0)'s un-tanh'ed output at
            # that point, and tanh can't run before ws -> same-pool
            # allocation would deadlock the PE stream.
            ws_sb = const_pool.tile([BL, D], f32r)
            id32r = const_pool.tile([BL, BL], f32r)
            nc.vector.tensor_copy(out=id32r, in_=id32[0:BL, 0:BL])
            wst_sb = []

            def emit_ws():
                ws_psum = [ctx_ps.tile([BL, 512], f32, tag="ctx", name=f"ws_psum{eh}")
                           for eh in range(2)]
                for dk in range(DC):
                    for eh in range(2):
                        nc.tensor.matmul(
                            ws_psum[eh], dect_sb[:, dk, :],
                            wbt_sb[dk // 2][:, dk % 2, eh * 512:(eh + 1) * 512],
                            start=(dk == 0), stop=(dk == DC - 1),
                        )
                for eh in range(2):
                    nc.scalar.copy(out=ws_sb[:, eh * 512:(eh + 1) * 512],
                                   in_=ws_psum[eh])
                # transpose ws -> wst chunks [128, BL] (tiny, K=BL transpose)
                for dcn in range(DC):
                    tp = sc_ps.tile([128, ST], f32r, tag="sc", name="tp_ws")
                    nc.tensor.transpose(
                        tp[:, 0:BL], ws_sb[0:BL, dcn * 128:(dcn + 1) * 128], id32r
                    )
                    w = const_pool.tile([128, BL], f32, tag="wst_sb", bufs=DC,
                                        name=f"wst{dcn}")
                    nc.vector.tensor_copy(out=w, in_=tp[:, 0:BL])
                    wst_sb.append(w)

            # ---- main loop over (batch, s-tile) ----
            # The (exp-columnize + ctx) block of tile t is emitted after tile
            # t+1's WH/score work: the PE would otherwise idle ~1us per tile
            # waiting for ACT's exp. `pending` carries tile t's closure.
            state = {}   # per-b: exp_all, zparts, ctx
            pending = [] # [(bj, st, sub_big)]

            def emit_ctx(bj, st, sub_big):
                s0 = st * ST
                exp_all = state[bj]["exp_all"]
                # one matmul yields all NSUB exp columns: rows 0..3 of
                # exp_all hold the exp row at 128-element shifts (copied by
                # tiny sync-queue DMAs below), so lhsT[n, m] = exp[s0+128n+m]
                # and the basis rhs picks ex[:, n] = column n.
                ex = ex_ps.tile([128, NSUB], f32, tag="wh", name="ex")
                nc.tensor.matmul(
                    ex, exp_all[0:32, s0:s0 + 128], e0f16,
                    start=True, stop=True,
                )
                ext = row_pool.tile([128, NSUB], bf16, tag="ext", bufs=3)
                nc.vector.tensor_copy(out=ext, in_=ex)
                # per-tile psum group, folded into the SBUF accumulator by DVE
                ctx_acc = state[bj]["ctx_acc"]
                for eh in range(2):
                    ctx_t = ctx_ps.tile([1, 512], f32, tag="ctx", name="ctx_t")
                    for j in range(NSUB):
                        nc.tensor.matmul(
                            ctx_t,
                            ext[:, j:j + 1],
                            sub_big[:, j, eh * 512:(eh + 1) * 512],
                            start=(j == 0), stop=(j == NSUB - 1),
                        )
                    sl = ctx_acc[0:1, eh * 512:(eh + 1) * 512]
                    nc.vector.tensor_add(out=sl, in0=sl, in1=ctx_t)

            def finish_batch(bj):
                z = row_pool.tile([1, 1], f32, tag="z")
                nc.vector.reduce_sum(out=z, in_=state[bj]["zparts"], axis=X)
                rz = row_pool.tile([1, 1], f32, tag="rz")
                nc.vector.reciprocal(out=rz, in_=z)
                ctx_sb = row_pool.tile([1, E2], f32, tag="ctx_sb")
                nc.vector.tensor_scalar_mul(
                    out=ctx_sb, in0=state[bj]["ctx_acc"], scalar1=rz,
                )
                nc.sync.dma_start(out=outp[bj:bj + 1, :], in_=ctx_sb)

            emit_ws()

            for bj in range(BL):
                # [32, S] so the row->column move can be a K=32 matmul against
                # e0 (rows 1-31 are zero); only row 0 holds exp scores.
                # fp16 is safe here: scores are bounded well below fp16's
                # exp-overflow point (|score| <= ~5 for randn-scale inputs,
                # overflow needs >11).
                exp_all = row_pool.tile([32, S], f16, tag="exp_all")
                nc.vector.memset(exp_all, 0.0)
                zparts = row_pool.tile([1, NST], f32, tag="zparts")
                ctx_acc = row_pool.tile([1, E2], f32, tag="ctx_acc")
                nc.vector.memset(ctx_acc, 0.0)
                state[bj] = dict(exp_all=exp_all, zparts=zparts, ctx_acc=ctx_acc)

                for st in range(NST):
                    s0 = st * ST
                    sub_big = subs_cache.pop((bj, st), None)
                    if sub_big is None:
                        sub_big = load_subs(bj, st)
                    et8 = enct_cache.pop((bj, st), None)
                    if et8 is None:
                        et8 = load_enct(bj, st)

                    # WH^T + tanh + score, d-chunks in pairs. The c-loop is
                    # outermost so consecutive matmuls alternate PSUM banks
                    # and re-read the same et8 chunk while it's hot; the
                    # tanh+score block of pair dp is deferred until after
                    # pair dp+1's DR groups so the PE never waits on ACT.
                    def emit_dr_pair(dp):
                        whs = [wh_ps.tile([128, ST], f32, tag="wh", name=f"wh{dd}")
                               for dd in range(2)]
                        for c in range(EC2):
                            for dd in range(2):
                                dcn = dp * 2 + dd
                                nc.tensor.matmul(
                                    whs[dd],
                                    wct8_sb[:, c, :, dcn * 128:(dcn + 1) * 128],
                                    et8[:, c, :, :],
                                    start=(c == 0), stop=(c == EC2 - 1),
                                    perf_mode=DR,
                                )
                        return whs

                    def emit_tanh_score(dp, whs, sc):
                        for dd in range(2):
                            dcn = dp * 2 + dd
                            th = tanh_pool.tile([128, ST], bf16, tag="th", name="th")
                            nc.scalar.activation(
                                out=th, in_=whs[dd], func=TANH,
                                bias=wst_sb[dcn][:, bj:bj + 1], scale=DESCALE,
                            )
                            nc.tensor.matmul(
                                sc, wa_sb[:, dcn:dcn + 1], th,
                                start=(dcn == 0), stop=(dcn == DC - 1),
                            )

                    sc = sc_ps.tile([1, ST], f32, tag="sc")
                    prev = None
                    for dp in range(DC // 2):
                        whs = emit_dr_pair(dp)
                        if prev is not None:
                            emit_tanh_score(dp - 1, prev, sc)
                        prev = whs
                    emit_tanh_score(DC // 2 - 1, prev, sc)

                    # exp (+ per-tile partial of Z via accum_out), then rows
                    # 1..3 get the 128-shifted head of the same exp values so
                    # the columnize collapses to one K=32 matmul. ACT can't
                    # write at partition base 1 (BIR alignment) and gpsimd
                    # SWDGE descriptors would contend with the tile loads, so
                    # the copies ride the near-idle sync HWDGE queue; the
                    # one-tile ctx deferral hides their latency.
                    nc.scalar.activation(
                        out=exp_all[0:1, s0:s0 + ST], in_=sc, func=EXP,
                        accum_out=zparts[0:1, st:st + 1],
                    )
                    for r in range(1, NSUB):
                        nc.sync.dma_start(
                            out=exp_all[r:r + 1, s0:s0 + 128],
                            in_=exp_all[0:1, s0 + r * 128:s0 + (r + 1) * 128],
                        )

                    # deferred ctx of the previous tile
                    if pending:
                        emit_ctx(*pending.pop())
                    pending.append((bj, st, sub_big))

                    if st == NST - 1 and bj > 0:
                        # previous batch is fully accumulated once its last
                        # pending ctx ran (one tile ago) -> normalize + store
                        finish_batch(bj - 1)

            emit_ctx(*pending.pop())
            finish_batch(BL - 1)

    nc.finalize()
    return nc


def _cancel_quant(x, a, tie=TIE, topk=256):
    """Quantize x [rows, K] to fp8e4 such that the a-weighted rounding error
    sum_k a_k*(q-x)_k cancels to ~0 per row. Nearest rounding, then a greedy
    subset-sum over near-tie elements flips them to the adjacent grid point."""
    import ml_dtypes

    f8 = ml_dtypes.float8_e4m3
    x32 = np.ascontiguousarray(x, dtype=np.float32)
    a32 = np.asarray(a, dtype=np.float32)
    qn = x32.astype(f8)
    qnf = qn.astype(np.float32)
    qf = (2.0 * x32 - qnf).astype(f8)      # reflect across x -> adjacent grid pt
    qff = qf.astype(np.float32)
    dq = qnf - x32
    step = np.abs(qff - qnf)
    c = np.where(np.abs(dq) >= tie * step, (qff - qnf) * a32[None, :], 0.0)
    A = dq @ a32
    idx = np.argpartition(-np.abs(c), topk - 1, axis=1)[:, :topk]
    csel = np.take_along_axis(c, idx, axis=1)
    ord2 = np.argsort(-np.abs(csel), axis=1)
    idx = np.take_along_axis(idx, ord2, axis=1)
    csel = np.take_along_axis(csel, ord2, axis=1)
    R = A.copy()
    take = np.zeros(csel.shape, dtype=bool)
    for k in range(topk):
        ck = csel[:, k]
        better = np.abs(R + ck) < np.abs(R)
        R += np.where(better, ck, 0.0)
        take[:, k] = better
    flip = np.zeros(c.shape, dtype=bool)
    np.put_along_axis(flip, idx, take, axis=1)
    return np.where(flip, qf, qn)


def _prep_inputs(dec_prev_hidden, enc_outputs, Wb, Wc, Wa):
    import ml_dtypes

    bf16 = ml_dtypes.bfloat16

    dec_prev_hidden = np.asarray(dec_prev_hidden, dtype=np.float32)
    enc_outputs = np.ascontiguousarray(np.asarray(enc_outputs, dtype=np.float32))
    Wb = np.asarray(Wb, dtype=np.float32)
    Wc = np.asarray(Wc, dtype=np.float32)
    Wa = np.asarray(Wa, dtype=np.float32)

    wbt16 = np.ascontiguousarray(Wb.T).astype(bf16)              # [d, d2]
    # error-cancelling fp8 quantization: Wc rows weighted by Wa, enc rows
    # weighted by v = Wc_q @ Wa (the Wa-weighted column sums).
    wq = _cancel_quant(np.ascontiguousarray(Wc.T) * SC_W, Wa)    # [e, d] fp8
    v = (wq.astype(np.float32) @ Wa) / SC_W                      # [e]
    xq = _cancel_quant(
        (enc_outputs * SC_E).reshape(S * B, E2), v
    ).reshape(S, B, E2)                                          # fp8
    wct8 = np.ascontiguousarray(
        wq.reshape(EC2, 2, 128, D).transpose(2, 0, 1, 3)         # [128, EC2, 2, D]
    )
    dect = dec_prev_hidden.T                                     # [D, B]
    wa16 = np.ascontiguousarray(Wa.reshape(DC, 128).T).astype(bf16)  # [128, DC]

    in_maps = []
    for i in range(NCORES):
        bsl = slice(i * BL, (i + 1) * BL)
        shard = enc_outputs[:, bsl, :]                           # [S, BL, E2]
        enct8 = np.ascontiguousarray(
            xq[:, bsl, :]
            .transpose(2, 1, 0)                                  # [E2, BL, S]
            .reshape(EC2, 2, 128, BL, S)
            .transpose(3, 2, 0, 1, 4)                            # [BL,128,EC2,2,S]
        )
        dect_c = np.ascontiguousarray(
            dect[:, bsl].reshape(DC, 128, BL).transpose(1, 0, 2)
        ).astype(bf16)                                           # [128, DC, BL]
        in_maps.append({
            "enct8": enct8,
            "encn": np.ascontiguousarray(shard).astype(bf16),
            "dect": dect_c,
            "wbt": wbt16,
            "wct8": wct8,
            "wa2": wa16,
        })
    return in_maps


def _run(inputs, trace=False):
    from concourse.bass_utils import run_bass_kernel_spmd

    if "nc" not in _CACHE:
        _CACHE["nc"] = _build_nc()
    nc = _CACHE["nc"]
    in_maps = _prep_inputs(**inputs)
    res = run_bass_kernel_spmd(nc, in_maps, list(range(NCORES)), trace=trace)
    out = np.concatenate([res.results[i]["out"] for i in range(NCORES)], axis=0)
    return out[None, :, :].astype(np.float32), res


def kernel(dec_prev_hidden, enc_outputs, Wb, Wc, Wa):
    out, _ = _run(dict(
        dec_prev_hidden=dec_prev_hidden, enc_outputs=enc_outputs,
        Wb=Wb, Wc=Wc, Wa=Wa,
    ))
    return out


# revision 38
# speedup vs baseline: 1.0360x; 1.0178x over previous
"""Bahdanau attention on 8 Trainium2 NeuronCores (Bass/Tile).

Problem:  S=2048, B=32, D=1024, E2=1024
  ws  = dec @ Wb.T                       [B, D]
  WH  = enc @ Wc.T                       [S, B, D]
  sc  = tanh(WH + ws) . Wa               [S, B]
  at  = softmax(sc, axis=0)              [S, B]
  out = einsum('sb,sbe->be', at, enc)[None]   [1, B, 2E]

Sharding: data-parallel over batch B across 8 cores (4 batches/core);
Wb/Wc/Wa replicated. Softmax axis (S) stays core-local.

The WH matmul (17.2 GFLOP/core) is the PE bottleneck. It runs as fp8e4
DoubleRow passes, each contracting TWO 128-e chunks in the time a bf16
matmul contracts one (measured ~216ns per 512-col pass either way), so
the WH instruction stream halves: 32 passes per (batch, s-tile) vs 64.

Plain nearest-rounded fp8 on both operands lands at rel-err 2.2e-2,
over the 2e-2 budget: the ~6%/element e4m3 noise becomes ~0.025
score noise which the softmax turns into context error. Fix: host-side
WEIGHTED ERROR-CANCELLING QUANTIZATION. The score only feels rounding
error through weighted sums -- sum_d Wa_d * dWc[e,d] per e-row of Wc,
and sum_e v_e * denc[s,b,e] (v = Wc_q @ Wa) per (s,b) row of enc. After
nearest rounding, ~21 near-tie elements per row are flipped to their
adjacent fp8 grid point (greedy subset-sum) so those weighted error
sums cancel to ~0. This removes the score-aligned noise component at
~zero per-element cost: measured rel-err 1.24e-2 on the target inputs.
The fp8 scales (enc*4, Wc*32) dodge e4m3 subnormals; the /128 descale
is folded into the tanh activation's scale.

All other tensors are bf16 (enc natural layout for the context matmul,
Wb/Wa/dec), halving DMA vs fp32. Numerics: tanh output bf16, exp row
f16, context accumulated fp32 in PSUM and normalized by 1/Z at the end
(no max-subtraction: |score| <= ~5 for these inputs, exp fits f16).

Per-core dataflow per (batch, s-tile):
  - WH^T [d-chunk=128p, s'] = 4 fp8 DoubleRow passes (8 e-chunks) in PSUM
  - ACT: tanh(WH/128 + wsT[d,b]) via per-partition bias, scale=1/128
  - score row [1, s'] = sum_d WaChunk.T @ tanh_chunk  (PE, bf16)
  - exp on ACT, Z partials via activation accum_out
  - exp row -> column via a K=32 matmul against e0 (rows 1-31 zeroed)
  - context [1, e] += expCol.T @ enc_nat(bf16) on PE, scaled 1/Z per batch

Engines run their instruction streams in order, so emission order doubles
as a schedule: tile(0,0)'s enc DMAs are emitted before the bulky weight
DMAs to cover DMA latency at kernel start; the (columnize + ctx) block of
tile t is emitted after tile t+1's WH/score work so the PE never stalls
on ACT's exp.
"""

import numpy as np

S, B, D, E2 = 2048, 32, 1024, 1024
NCORES = 8
BL = B // NCORES          # batches per core
ST = 512                  # s-tile size
NST = S // ST             # s-tiles per batch
NSUB = ST // 128          # 128-row subtiles per s-tile
DC = D // 128             # d chunks
EC2 = E2 // 256           # e chunk-pairs (one DoubleRow pass each)

SC_E = 4.0                # fp8 enc pre-scale (power of 2, exact)
SC_W = 32.0               # fp8 Wc pre-scale
DESCALE = 1.0 / (SC_E * SC_W)
TIE = 0.45                # flip only near-tie roundings (|err| >= TIE*step)

_CACHE = {}


def _build_nc():
    import concourse.bacc as bacc
    import concourse.tile as tile
    from concourse import mybir
    from concourse.masks import make_identity

    f32 = mybir.dt.float32
    f32r = mybir.dt.float32r
    f16 = mybir.dt.float16
    bf16 = mybir.dt.bfloat16
    f8 = mybir.dt.float8e4
    DR = mybir.MatmulPerfMode.DoubleRow
    TANH = mybir.ActivationFunctionType.Tanh
    EXP = mybir.ActivationFunctionType.Exp
    X = mybir.AxisListType.X

    nc = bacc.Bacc()
    # enct8[b, p, c, i, s] = fp8q(enc[s, b, 256c+128i+p] * 4)
    enct8 = nc.declare_dram_parameter("enct8", [BL, 128, EC2, 2, S], f8, isOutput=False)
    encn = nc.declare_dram_parameter("encn", [S, BL, E2], bf16, isOutput=False)
    dect = nc.declare_dram_parameter("dect", [128, DC, BL], bf16, isOutput=False)
    wbt = nc.declare_dram_parameter("wbt", [D, D], bf16, isOutput=False)     # Wb.T
    # wct8[p, c, i, d] = fp8q(Wc.T[256c+128i+p, d] * 32)
    wct8 = nc.declare_dram_parameter("wct8", [128, EC2, 2, D], f8, isOutput=False)
    wa2 = nc.declare_dram_parameter("wa2", [128, DC], bf16, isOutput=False)  # Wa chunks
    outp = nc.declare_dram_parameter("out", [BL, E2], f32, isOutput=True)

    with tile.TileContext(nc) as tc:
        with (
            tc.tile_pool(name="const", bufs=1) as const_pool,
            tc.tile_pool(name="wbtp", bufs=1) as wbt_pool,
            tc.tile_pool(name="encn", bufs=3) as encn_pool,
            tc.tile_pool(name="enct", bufs=3) as enct_pool,
            tc.tile_pool(name="tanhp", bufs=4) as tanh_pool,
            tc.tile_pool(name="rows", bufs=2) as row_pool,
            tc.tile_pool(name="wh_ps", bufs=4, space="PSUM") as wh_ps,
            tc.tile_pool(name="sc_ps", bufs=2, space="PSUM") as sc_ps,
            tc.tile_pool(name="ctx_ps", bufs=2, space="PSUM") as ctx_ps,
        ):
            ex_ps = wh_ps  # columnize rides the wh rotation (same tag below)

            # ---- identity (fp32 via gpsimd) + fp16 e0 for the columnize ----
            id32 = const_pool.tile([128, 128], f32)
            make_identity(nc, id32)
            e0f16 = const_pool.tile([32, 1], f16)
            nc.vector.tensor_copy(out=e0f16, in_=id32[0:32, 0:1])

            # ---- PE clock warm-up ----
            # The PE ramps 0.65 -> 1.2 -> 2.4 GHz over ~3us of continuous
            # execution. Burn that ramp on dummy fp32 matmuls (4 cyc/row)
            # during the otherwise-idle startup DMA window so the ws chain
            # and tile(0,0) run at full clock.
            warm_ps = wh_ps.tile([128, ST], f32, tag="wh", name="warm")
            for _ in range(6):
                nc.tensor.matmul(warm_ps[:, 0:128], id32, id32,
                                 start=True, stop=True)

            # ---- emission helpers (order == per-engine schedule) ----
            # One fused dma_start per tile load: SWDGE descriptor generation
            # costs ~0.6us per dma_start on the GpSimd Q7, so batch the
            # subtiles into a single transfer with a 3/4-D access pattern.
            def load_subs(bj, st, pfx=""):
                s0 = st * ST
                sub_big = encn_pool.tile([128, NSUB, E2], bf16, tag="sub",
                                         name=f"sub{pfx}")
                nc.gpsimd.dma_start(
                    out=sub_big,
                    in_=encn[s0:s0 + ST, bj, :].rearrange("(j p) e -> p j e", p=128),
                )
                return sub_big

            def load_enct(bj, st, pfx=""):
                s0 = st * ST
                et8 = enct_pool.tile([128, EC2, 2, ST], f8, tag="et8", bufs=3,
                                     name=f"et8{pfx}")
                nc.gpsimd.dma_start(out=et8, in_=enct8[bj, :, :, :, s0:s0 + ST])
                return et8

            # DMA queue order tuned for startup, strictly by first PE need:
            # ws inputs (dect tiny, then wbt as one fused load), then the
            # WH-path tile(0,0) load, then Wc^T, then the ctx-path tile(0,0).
            dect_sb = const_pool.tile([128, DC, BL], bf16)
            nc.gpsimd.dma_start(out=dect_sb, in_=dect[:, :, :])
            # wbt in 4 chunked loads of 2 d-blocks: the first ws matmul only
            # waits ~512KB, and later chunks land while the PE consumes
            # earlier ones.
            wbt_sb = []
            for g in range(4):
                t = wbt_pool.tile([128, 2, D], bf16, tag="wbt_sb", bufs=4,
                                  name=f"wbt{g}")
                nc.gpsimd.dma_start(
                    out=t,
                    in_=wbt[g * 256:(g + 1) * 256, :]
                    .rearrange("(c p) d -> p c d", p=128),
                )
                wbt_sb.append(t)
            enct_cache = {(0, 0): load_enct(0, 0, pfx="00")}
            wct8_sb = const_pool.tile([128, EC2, 2, D], f8)
            nc.gpsimd.dma_start(out=wct8_sb, in_=wct8[:, :, :, :])
            subs_cache = {(0, 0): load_subs(0, 0, pfx="00")}
            wa_sb = const_pool.tile([128, DC], bf16)
            nc.gpsimd.dma_start(out=wa_sb, in_=wa2[:, :])

            # ---- ws = dec @ Wb.T -> wsT [d2-chunk, b] for the tanh bias ----
            # Emitted AFTER tile(0,0)'s WH DoubleRow groups (see emit_wh) so
            # the PE crunches tile(0,0) while the ws-path DMAs land. The ws
            # psum rides the ctx pool and the transpose scratch the sc pool:
            # all four wh-pool bufs hold tile(0,0)'s un-tanh'ed output at
            # that point, and tanh can't run before ws -> same-pool
            # allocation would deadlock the PE stream.
            ws_sb = const_pool.tile([BL, D], f32r)
            id32r = const_pool.tile([BL, BL], f32r)
            nc.vector.tensor_copy(out=id32r, in_=id32[0:BL, 0:BL])
            wst_sb = []

            def emit_ws():
                ws_psum = [ctx_ps.tile([BL, 512], f32, tag="ctx", name=f"ws_psum{eh}")
                           for eh in range(2)]
                for dk in range(DC):
                    for eh in range(2):
                        nc.tensor.matmul(
                            ws_psum[eh], dect_sb[:, dk, :],
                            wbt_sb[dk // 2][:, dk % 2, eh * 512:(eh + 1) * 512],
                            start=(dk == 0), stop=(dk == DC - 1),
                        )
                for eh in range(2):
                    nc.scalar.copy(out=ws_sb[:, eh * 512:(eh + 1) * 512],
                                   in_=ws_psum[eh])
                # transpose ws -> wst chunks [128, BL] (tiny, K=BL transpose)
                for dcn in range(DC):
                    tp = sc_ps.tile([128, ST], f32r, tag="sc", name="tp_ws")
                    nc.tensor.transpose(
                        tp[:, 0:BL], ws_sb[0:BL, dcn * 128:(dcn + 1) * 128], id32r
                    )
                    w = const_pool.tile([128, BL], f32, tag="wst_sb", bufs=DC,
                                        name=f"wst{dcn}")
                    nc.vector.tensor_copy(out=w, in_=tp[:, 0:BL])
                    wst_sb.append(w)

            # ---- main loop over (batch, s-tile) ----
            # The (exp-columnize + ctx) block of tile t is emitted after tile
            # t+1's WH/score work: the PE would otherwise idle ~1us per tile
            # waiting for ACT's exp. `pending` carries tile t's closure.
            state = {}   # per-b: exp_all, zparts, ctx
            pending = [] # [(bj, st, sub_big)]

            def emit_ctx(bj, st, sub_big):
                s0 = st * ST
                exp_all = state[bj]["exp_all"]
                ex = ex_ps.tile([128, NSUB], f32, tag="wh", name="ex")
                for j in range(NSUB):
                    nc.tensor.matmul(
                        ex[:, j:j + 1],
                        exp_all[0:32, s0 + j * 128:s0 + (j + 1) * 128],
                        e0f16,
                        start=True, stop=True,
                    )
                ext = row_pool.tile([128, NSUB], bf16, tag="ext", bufs=3)
                nc.vector.tensor_copy(out=ext, in_=ex)
                # per-tile psum group, folded into the SBUF accumulator by DVE
                ctx_acc = state[bj]["ctx_acc"]
                for eh in range(2):
                    ctx_t = ctx_ps.tile([1, 512], f32, tag="ctx", name="ctx_t")
                    for j in range(NSUB):
                        nc.tensor.matmul(
                            ctx_t,
                            ext[:, j:j + 1],
                            sub_big[:, j, eh * 512:(eh + 1) * 512],
                            start=(j == 0), stop=(j == NSUB - 1),
                        )
                    sl = ctx_acc[0:1, eh * 512:(eh + 1) * 512]
                    nc.vector.tensor_add(out=sl, in0=sl, in1=ctx_t)

            def finish_batch(bj):
                z = row_pool.tile([1, 1], f32, tag="z")
                nc.vector.reduce_sum(out=z, in_=state[bj]["zparts"], axis=X)
                rz = row_pool.tile([1, 1], f32, tag="rz")
                nc.vector.reciprocal(out=rz, in_=z)
                ctx_sb = row_pool.tile([1, E2], f32, tag="ctx_sb")
                nc.vector.tensor_scalar_mul(
                    out=ctx_sb, in0=state[bj]["ctx_acc"], scalar1=rz,
                )
                nc.sync.dma_start(out=outp[bj:bj + 1, :], in_=ctx_sb)

            emit_ws()

            for bj in range(BL):
                # [32, S] so the row->column move can be a K=32 matmul against
                # e0 (rows 1-31 are zero); only row 0 holds exp scores.
                # fp16 is safe here: scores are bounded well below fp16's
                # exp-overflow point (|score| <= ~5 for randn-scale inputs,
                # overflow needs >11).
                exp_all = row_pool.tile([32, S], f16, tag="exp_all")
                nc.vector.memset(exp_all, 0.0)
                zparts = row_pool.tile([1, NST], f32, tag="zparts")
                ctx_acc = row_pool.tile([1, E2], f32, tag="ctx_acc")
                nc.vector.memset(ctx_acc, 0.0)
                state[bj] = dict(exp_all=exp_all, zparts=zparts, ctx_acc=ctx_acc)

                for st in range(NST):
                    s0 = st * ST
                    sub_big = subs_cache.pop((bj, st), None)
                    if sub_big is None:
                        sub_big = load_subs(bj, st)
                    et8 = enct_cache.pop((bj, st), None)
                    if et8 is None:
                        et8 = load_enct(bj, st)

                    # WH^T + tanh + score, d-chunks in pairs. The c-loop is
                    # outermost so consecutive matmuls alternate PSUM banks
                    # and re-read the same et8 chunk while it's hot; the
                    # tanh+score block of pair dp is deferred until after
                    # pair dp+1's DR groups so the PE never waits on ACT.
                    def emit_dr_pair(dp):
                        whs = [wh_ps.tile([128, ST], f32, tag="wh", name=f"wh{dd}")
                               for dd in range(2)]
                        for c in range(EC2):
                            for dd in range(2):
                                dcn = dp * 2 + dd
                                nc.tensor.matmul(
                                    whs[dd],
                                    wct8_sb[:, c, :, dcn * 128:(dcn + 1) * 128],
                                    et8[:, c, :, :],
                                    start=(c == 0), stop=(c == EC2 - 1),
                                    perf_mode=DR,
                                )
                        return whs

                    def emit_tanh_score(dp, whs, sc):
                        for dd in range(2):
                            dcn = dp * 2 + dd
                            th = tanh_pool.tile([128, ST], bf16, tag="th", name="th")
                            nc.scalar.activation(
                                out=th, in_=whs[dd], func=TANH,
                                bias=wst_sb[dcn][:, bj:bj + 1], scale=DESCALE,
                            )
                            nc.tensor.matmul(
                                sc, wa_sb[:, dcn:dcn + 1], th,
                                start=(dcn == 0), stop=(dcn == DC - 1),
                            )

                    sc = sc_ps.tile([1, ST], f32, tag="sc")
                    prev = None
                    for dp in range(DC // 2):
                        whs = emit_dr_pair(dp)
                        if prev is not None:
                            emit_tanh_score(dp - 1, prev, sc)
                        prev = whs
                    emit_tanh_score(DC // 2 - 1, prev, sc)

                    # exp (+ per-tile partial of Z via accum_out)
                    nc.scalar.activation(
                        out=exp_all[0:1, s0:s0 + ST], in_=sc, func=EXP,
                        accum_out=zparts[0:1, st:st + 1],
                    )

                    # deferred ctx of the previous tile
                    if pending:
                        emit_ctx(*pending.pop())
                    pending.append((bj, st, sub_big))

                    if st == NST - 1 and bj > 0:
                        # previous batch is fully accumulated once its last
                        # pending ctx ran (one tile ago) -> normalize + store
                        finish_batch(bj - 1)

            emit_ctx(*pending.pop())
            finish_batch(BL - 1)

    nc.finalize()
    return nc


def _cancel_quant(x, a, tie=TIE, topk=256):
    """Quantize x [rows, K] to fp8e4 such that the a-weighted rounding error
    sum_k a_k*(q-x)_k cancels to ~0 per row. Nearest rounding, then a greedy
    subset-sum over near-tie elements flips them to the adjacent grid point."""
    import ml_dtypes

    f8 = ml_dtypes.float8_e4m3
    x32 = np.ascontiguousarray(x, dtype=np.float32)
    a32 = np.asarray(a, dtype=np.float32)
    qn = x32.astype(f8)
    qnf = qn.astype(np.float32)
    qf = (2.0 * x32 - qnf).astype(f8)      # reflect across x -> adjacent grid pt
    qff = qf.astype(np.float32)
    dq = qnf - x32
    step = np.abs(qff - qnf)
    c = np.where(np.abs(dq) >= tie * step, (qff - qnf) * a32[None, :], 0.0)
    A = dq @ a32
    idx = np.argpartition(-np.abs(c), topk - 1, axis=1)[:, :topk]
    csel = np.take_along_axis(c, idx, axis=1)
    ord2 = np.argsort(-np.abs(csel), axis=1)
    idx = np.take_along_axis(idx, ord2, axis=1)
    csel = np.take_along_axis(csel, ord2, axis=1)
    R = A.copy()
    take = np.zeros(csel.shape, dtype=bool)
    for k in range(topk):
        ck = csel[:, k]
        better = np.abs(R + ck) < np.abs(R)
        R += np.where(better, ck, 0.0)
        take[:, k] = better
    flip = np.zeros(c.shape, dtype=bool)
    np.put_along_axis(flip, idx, take, axis=1)
    return np.where(flip, qf, qn)


def _prep_inputs(dec_prev_hidden, enc_outputs, Wb, Wc, Wa):
    import ml_dtypes

    bf16 = ml_dtypes.bfloat16

    dec_prev_hidden = np.asarray(dec_prev_hidden, dtype=np.float32)
    enc_outputs = np.ascontiguousarray(np.asarray(enc_outputs, dtype=np.float32))
    Wb = np.asarray(Wb, dtype=np.float32)
    Wc = np.asarray(Wc, dtype=np.float32)
    Wa = np.asarray(Wa, dtype=np.float32)

    wbt16 = np.ascontiguousarray(Wb.T).astype(bf16)              # [d, d2]
    # error-cancelling fp8 quantization: Wc rows weighted by Wa, enc rows
    # weighted by v = Wc_q @ Wa (the Wa-weighted column sums).
    wq = _cancel_quant(np.ascontiguousarray(Wc.T) * SC_W, Wa)    # [e, d] fp8
    v = (wq.astype(np.float32) @ Wa) / SC_W                      # [e]
    xq = _cancel_quant(
        (enc_outputs * SC_E).reshape(S * B, E2), v
    ).reshape(S, B, E2)                                          # fp8
    wct8 = np.ascontiguousarray(
        wq.reshape(EC2, 2, 128, D).transpose(2, 0, 1, 3)         # [128, EC2, 2, D]
    )
    dect = dec_prev_hidden.T                                     # [D, B]
    wa16 = np.ascontiguousarray(Wa.reshape(DC, 128).T).astype(bf16)  # [128, DC]

    in_maps = []
    for i in range(NCORES):
        bsl = slice(i * BL, (i + 1) * BL)
        shard = enc_outputs[:, bsl, :]                           # [S, BL, E2]
        enct8 = np.ascontiguousarray(
            xq[:, bsl, :]
            .transpose(2, 1, 0)                                  # [E2, BL, S]
            .reshape(EC2, 2, 128, BL, S)
            .transpose(3, 2, 0, 1, 4)                            # [BL,128,EC2,2,S]
        )
        dect_c = np.ascontiguousarray(
            dect[:, bsl].reshape(DC, 128, BL).transpose(1, 0, 2)
        ).astype(bf16)                                           # [128, DC, BL]
        in_maps.append({
            "enct8": enct8,
            "encn": np.ascontiguousarray(shard).astype(bf16),
            "dect": dect_c,
            "wbt": wbt16,
            "wct8": wct8,
            "wa2": wa16,
        })
    return in_maps


def _run(inputs, trace=False):
    from concourse.bass_utils import run_bass_kernel_spmd

    if "nc" not in _CACHE:
        _CACHE["nc"] = _build_nc()
    nc = _CACHE["nc"]
    in_maps = _prep_inputs(**inputs)
    res = run_bass_kernel_spmd(nc, in_maps, list(range(NCORES)), trace=trace)
    out = np.concatenate([res.results[i]["out"] for i in range(NCORES)], axis=0)
    return out[None, :, :].astype(np.float32), res


def kernel(dec_prev_hidden, enc_outputs, Wb, Wc, Wa):
    out, _ = _run(dict(
        dec_prev_hidden=dec_prev_hidden, enc_outputs=enc_outputs,
        Wb=Wb, Wc=Wc, Wa=Wa,
    ))
    return out
